# revision 9
# baseline (speedup 1.0000x reference)
"""3x3 median filter (reflect padding) on Trainium2, 8-core data parallel.

Input  x: (4, 3, 1024, 1024) float32
Output  : (4, 3, 1024, 1024) float32  (Kornia MedianBlur semantics)

Strategy:
  - Host: convert to fp16 (DVE tensor_tensor runs 2 elem/cycle on 16-bit
    data; max err ~2^-11 << 2e-2 tolerance), reflect-pad H and W by 1,
    shard H across 8 cores with 1-row halo.
  - Device (per core): separable median network.  The vector engine is
    the only engine with tensor-tensor min/max, so it is the bottleneck;
    two of the 18 ops/pixel are offloaded using sum identities computed
    on the idle tensor + scalar engines:
        M   = max(T0,T1) = T0 + T1 - min(T0,T1)
        mid = med3(col)  = T0 + T1 + T2 - lo - hi
    PE accumulates the sums via +/-identity matmuls into PSUM (512-wide
    strips), ACT copies PSUM -> SBUF fp16.  DVE does the remaining 15
    min/max ops per pixel.
  - Images processed in chunks [2,4,4,2] concatenated along the free dim
    (amortizes the ~150-cycle/op overhead; junk junction columns never
    read).  DVE ops emitted in a stall-minimizing order (lo/pa/A before
    hi, which waits on the ACT-produced M).
"""

import sys

sys.path.insert(0, "/opt/trn_rl_repo")

import numpy as np

B, C, H, W = 4, 3, 1024, 1024
NIMG = B * C            # 12
NCORES = 8
ROWS_PER_CORE = H // NCORES   # 128
WP = W + 2              # 1026 padded width
HP_CORE = ROWS_PER_CORE + 2   # 130 padded rows per core
CHUNKS = (2, 4, 4, 2)   # images per chunk, sum = NIMG
GMAX = max(CHUNKS)
NMAX = GMAX * WP        # 4104

_PROGRAM = None
LAST_RESULT = None


def _build_program():
    import concourse.bacc as bacc
    import concourse.tile as tile
    import concourse.mybir as mybir
    from concourse.bass import MemorySpace
    from contextlib import ExitStack

    f16 = mybir.dt.float16
    f32 = mybir.dt.float32
    mn = mybir.AluOpType.min
    mx = mybir.AluOpType.max
    COPYF = mybir.ActivationFunctionType.Copy

    nc = bacc.Bacc("TRN2", target_bir_lowering=False, debug=False,
                   num_devices=NCORES)
    x = nc.dram_tensor("x", [NIMG, HP_CORE, WP], f16, kind="ExternalInput").ap()
    wid = nc.dram_tensor("wid", [2, 128, 128], f16, kind="ExternalInput").ap()
    y = nc.dram_tensor("y", [NIMG, ROWS_PER_CORE, W], f16,
                       kind="ExternalOutput").ap()

    P = ROWS_PER_CORE  # 128 partitions

    with tile.TileContext(nc) as tc, ExitStack() as ctx:
        pool = ctx.enter_context(tc.tile_pool(name="p", bufs=2))
        cpool = ctx.enter_context(tc.tile_pool(name="c", bufs=1))
        psum = ctx.enter_context(
            tc.tile_pool(name="ps", bufs=1, space=MemorySpace.PSUM))

        Iw = cpool.tile([P, P], f16, tag="Iw")
        NIw = cpool.tile([P, P], f16, tag="NIw")
        nc.sync.dma_start(Iw[:], wid[0])
        nc.sync.dma_start(NIw[:], wid[1])

        def tt(dst, a, b, op):
            nc.vector.tensor_tensor(dst, a, b, op=op)

        def pe_sum(dst_sbuf, n, terms, bank_base):
            """dst_sbuf[:, 0:n] = sum(sign*t for sign, t in terms), via PE
            accumulation in 512-wide PSUM strips + ACT copy to SBUF fp16."""
            strips = [(s, min(512, n - s)) for s in range(0, n, 512)]
            for si, (s, w) in enumerate(strips):
                bank = (bank_base + si) % 8
                ps = psum.tile([P, 512], f32, tag=f"ps{bank}", name=f"ps{bank}")
                for ti, (sign, t) in enumerate(terms):
                    nc.tensor.matmul(ps[:, 0:w], (Iw if sign > 0 else NIw)[:],
                                     t[:, s:s + w],
                                     start=(ti == 0), stop=(ti == len(terms) - 1))
                nc.scalar.activation(dst_sbuf[:, s:s + w], ps[:, 0:w], COPYF)

        i0 = 0
        for c, G in enumerate(CHUNKS):
            last = c == len(CHUNKS) - 1
            N = G * WP
            Bufs = [pool.tile([P, NMAX], f16, tag=f"B{k}", name=f"B{k}")
                    for k in range(8)]
            T0, T1, T2 = Bufs[0], Bufs[1], Bufs[2]
            for g in range(G):
                s = slice(g * WP, (g + 1) * WP)
                nc.gpsimd.dma_start(T0[:, s], x[i0 + g, 0:P, :])
                nc.scalar.dma_start(T1[:, s], x[i0 + g, 1:P + 1, :])
                nc.sync.dma_start(T2[:, s], x[i0 + g, 2:P + 2, :])

            if False:
                # head chunk: pure-DVE 18-op network -- no cross-engine
                # (PE/ACT) dependencies while the input DMA is still
                # streaming in, so the DVE pipeline never stalls on the
                # M_/mid zigzag before other chunks exist to fill it.
                m_, M_ = Bufs[3], Bufs[4]
                tt(m_[:, 0:N], T0[:, 0:N], T1[:, 0:N], mn)
                tt(M_[:, 0:N], T0[:, 0:N], T1[:, 0:N], mx)
                lo, mm, hi = Bufs[0], Bufs[1], Bufs[5]
                tt(lo[:, 0:N], m_[:, 0:N], T2[:, 0:N], mn)
                tt(mm[:, 0:N], M_[:, 0:N], T2[:, 0:N], mn)
                tt(hi[:, 0:N], M_[:, 0:N], T2[:, 0:N], mx)
                mid = Bufs[2]
                tt(mid[:, 0:N], m_[:, 0:N], mm[:, 0:N], mx)
                pa, A = Bufs[3], Bufs[7]
                tt(pa[:, 0:N - 1], lo[:, 0:N - 1], lo[:, 1:N], mx)
                tt(A[:, 0:N - 2], pa[:, 0:N - 2], lo[:, 2:N], mx)
                pc, Cm = Bufs[4], Bufs[1]
                tt(pc[:, 0:N - 1], hi[:, 0:N - 1], hi[:, 1:N], mn)
                tt(Cm[:, 0:N - 2], pc[:, 0:N - 2], hi[:, 2:N], mn)
                pm, pM = Bufs[0], Bufs[4]
                tt(pm[:, 0:N - 1], mid[:, 0:N - 1], mid[:, 1:N], mn)
                tt(pM[:, 0:N - 1], mid[:, 0:N - 1], mid[:, 1:N], mx)
                t2, Bm = Bufs[5], Bufs[3]
                tt(t2[:, 0:N - 2], pM[:, 0:N - 2], mid[:, 2:N], mn)
                tt(Bm[:, 0:N - 2], pm[:, 0:N - 2], t2[:, 0:N - 2], mx)
                m1, M1 = Bufs[2], Bufs[4]
                t3, out = Bufs[0], Bufs[5]
            else:
                # T0/T1/T2 stay live until mid's PE terms read them.
                m_, M_, lo = Bufs[3], Bufs[4], Bufs[5]
                tt(m_[:, 0:N], T0[:, 0:N], T1[:, 0:N], mn)
                pe_sum(M_, N, [(1, T0), (1, T1), (-1, m_)], bank_base=0)
                tt(lo[:, 0:N], m_[:, 0:N], T2[:, 0:N], mn)
                # A-branch needs only lo: keeps DVE busy while PE/ACT make M_.
                pa, A = Bufs[3], Bufs[7]      # m_ dead after lo + M_ PE term
                tt(pa[:, 0:N - 1], lo[:, 0:N - 1], lo[:, 1:N], mx)
                tt(A[:, 0:N - 2], pa[:, 0:N - 2], lo[:, 2:N], mx)
                hi = Bufs[3]                  # pa dead after A
                tt(hi[:, 0:N], M_[:, 0:N], T2[:, 0:N], mx)
                mid = Bufs[6]
                pe_sum(mid, N, [(1, T0), (1, T1), (1, T2), (-1, lo), (-1, hi)],
                       bank_base=4)

                pc, Cm = Bufs[0], Bufs[1]     # T0, T1 dead after mid PE terms
                tt(pc[:, 0:N - 1], hi[:, 0:N - 1], hi[:, 1:N], mn)
                tt(Cm[:, 0:N - 2], pc[:, 0:N - 2], hi[:, 2:N], mn)
                pm, pM = Bufs[2], Bufs[0]     # T2 dead; pc dead after Cm
                tt(pm[:, 0:N - 1], mid[:, 0:N - 1], mid[:, 1:N], mn)
                tt(pM[:, 0:N - 1], mid[:, 0:N - 1], mid[:, 1:N], mx)
                t2, Bm = Bufs[5], Bufs[4]     # lo dead after pa/A + mid PE; M_ dead after hi
                tt(t2[:, 0:N - 2], pM[:, 0:N - 2], mid[:, 2:N], mn)
                tt(Bm[:, 0:N - 2], pm[:, 0:N - 2], t2[:, 0:N - 2], mx)

                m1, M1 = Bufs[6], Bufs[2]     # mid dead after t2; pm dead
                t3, out = Bufs[3], Bufs[5]    # hi dead after Cm; t2 dead
            if last:
                for g in range(G):
                    s = slice(g * WP, g * WP + W)
                    tt(m1[:, s], A[:, s], Bm[:, s], mn)
                    tt(M1[:, s], A[:, s], Bm[:, s], mx)
                    tt(t3[:, s], M1[:, s], Cm[:, s], mn)
                    tt(out[:, s], m1[:, s], t3[:, s], mx)
                    eng = (nc.gpsimd, nc.scalar, nc.sync)[g % 3]
                    eng.dma_start(y[i0 + g], out[:, s])
            else:
                tt(m1[:, 0:N - 2], A[:, 0:N - 2], Bm[:, 0:N - 2], mn)
                tt(M1[:, 0:N - 2], A[:, 0:N - 2], Bm[:, 0:N - 2], mx)
                tt(t3[:, 0:N - 2], M1[:, 0:N - 2], Cm[:, 0:N - 2], mn)
                tt(out[:, 0:N - 2], m1[:, 0:N - 2], t3[:, 0:N - 2], mx)
                for g in range(G):
                    eng = (nc.gpsimd, nc.scalar, nc.sync)[g % 3]
                    eng.dma_start(y[i0 + g], out[:, g * WP: g * WP + W])
            i0 += G

    nc.compile()
    return nc


def _get_program():
    global _PROGRAM
    if _PROGRAM is None:
        _PROGRAM = _build_program()
    return _PROGRAM


def kernel(x):
    global LAST_RESULT
    from concourse.bass_utils import run_bass_kernel_spmd
    import os

    x16 = np.asarray(x).astype(np.float16).reshape(NIMG, H, W)
    xp = np.pad(x16, ((0, 0), (1, 1), (1, 1)), mode="reflect")
    wid = np.stack([np.eye(128, dtype=np.float16),
                    -np.eye(128, dtype=np.float16)])
    in_maps = [
        {"x": np.ascontiguousarray(
            xp[:, ROWS_PER_CORE * k: ROWS_PER_CORE * k + HP_CORE, :]),
         "wid": wid}
        for k in range(NCORES)
    ]
    nc = _get_program()
    trace = bool(int(os.environ.get("MEDIAN_TRACE", "0")))
    res = run_bass_kernel_spmd(nc, in_maps, list(range(NCORES)), trace=trace)
    LAST_RESULT = res
    out = np.concatenate([res.results[k]["y"] for k in range(NCORES)], axis=1)
    return out.reshape(B, C, H, W).astype(np.float32)


# revision 11
# speedup vs baseline: 1.0075x; 1.0075x over previous
"""3x3 median filter (reflect padding) on Trainium2, 8-core data parallel.

Input  x: (4, 3, 1024, 1024) float32
Output  : (4, 3, 1024, 1024) float32  (Kornia MedianBlur semantics)

Strategy:
  - Host: convert to fp16 (DVE tensor_tensor runs 2 elem/cycle on 16-bit
    data; max err ~2^-11 << 2e-2 tolerance), reflect-pad H and W by 1,
    shard H across 8 cores with 1-row halo.
  - Device (per core): separable median network.  The vector engine is
    the only engine with tensor-tensor min/max, so it is the bottleneck;
    two of the 18 ops/pixel are offloaded using sum identities computed
    on the idle tensor + scalar engines:
        M   = max(T0,T1) = T0 + T1 - min(T0,T1)
        mid = med3(col)  = T0 + T1 + T2 - lo - hi
    PE accumulates the sums via +/-identity matmuls into PSUM (512-wide
    strips), ACT copies PSUM -> SBUF fp16.  DVE does the remaining 15
    min/max ops per pixel.
  - Images processed in chunks [2,4,4,2] concatenated along the free dim
    (amortizes the ~150-cycle/op overhead; junk junction columns never
    read).  DVE ops emitted in a stall-minimizing order (lo/pa/A before
    hi, which waits on the ACT-produced M).
"""

import sys

sys.path.insert(0, "/opt/trn_rl_repo")

import numpy as np

B, C, H, W = 4, 3, 1024, 1024
NIMG = B * C            # 12
NCORES = 8
ROWS_PER_CORE = H // NCORES   # 128
WP = W + 2              # 1026 padded width
HP_CORE = ROWS_PER_CORE + 2   # 130 padded rows per core
CHUNKS = (2, 4, 4, 2)   # images per chunk, sum = NIMG
GMAX = max(CHUNKS)
NMAX = GMAX * WP        # 4104

_PROGRAM = None
LAST_RESULT = None


def _build_program():
    import concourse.bacc as bacc
    import concourse.tile as tile
    import concourse.mybir as mybir
    from concourse.bass import MemorySpace
    from contextlib import ExitStack

    f16 = mybir.dt.float16
    f32 = mybir.dt.float32
    mn = mybir.AluOpType.min
    mx = mybir.AluOpType.max
    COPYF = mybir.ActivationFunctionType.Copy

    nc = bacc.Bacc("TRN2", target_bir_lowering=False, debug=False,
                   num_devices=NCORES)
    x = nc.dram_tensor("x", [NIMG, HP_CORE, WP], f16, kind="ExternalInput").ap()
    wid = nc.dram_tensor("wid", [2, 128, 128], f16, kind="ExternalInput").ap()
    y = nc.dram_tensor("y", [NIMG, ROWS_PER_CORE, W], f16,
                       kind="ExternalOutput").ap()

    P = ROWS_PER_CORE  # 128 partitions

    with tile.TileContext(nc) as tc, ExitStack() as ctx:
        pool = ctx.enter_context(tc.tile_pool(name="p", bufs=2))
        cpool = ctx.enter_context(tc.tile_pool(name="c", bufs=1))
        psum = ctx.enter_context(
            tc.tile_pool(name="ps", bufs=1, space=MemorySpace.PSUM))

        Iw = cpool.tile([P, P], f16, tag="Iw")
        NIw = cpool.tile([P, P], f16, tag="NIw")
        nc.sync.dma_start(Iw[:], wid[0])
        nc.sync.dma_start(NIw[:], wid[1])

        def tt(dst, a, b, op):
            nc.vector.tensor_tensor(dst, a, b, op=op)

        def pe_sum(dst_sbuf, n, terms, bank_base):
            """dst_sbuf[:, 0:n] = sum(sign*t for sign, t in terms), via PE
            accumulation in 512-wide PSUM strips + ACT copy to SBUF fp16."""
            strips = [(s, min(512, n - s)) for s in range(0, n, 512)]
            for si, (s, w) in enumerate(strips):
                bank = (bank_base + si) % 7
                ps = psum.tile([P, 512], f32, tag=f"ps{bank}", name=f"ps{bank}")
                for ti, (sign, t) in enumerate(terms):
                    nc.tensor.matmul(ps[:, 0:w], (Iw if sign > 0 else NIw)[:],
                                     t[:, s:s + w],
                                     start=(ti == 0), stop=(ti == len(terms) - 1))
                nc.scalar.activation(dst_sbuf[:, s:s + w], ps[:, 0:w], COPYF)

        def pe_keepalive(dep):
            """Tiny matmul reading a just-produced DVE tile: executes right
            after `dep` is written, keeping the PE's HAM activity window from
            re-throttling the clock to 1.2 GHz during long DVE stretches."""
            ka = psum.tile([P, 512], f32, tag="ps7", name="ps7")
            nc.tensor.matmul(ka[:, 0:16], Iw[:], dep[:, 0:16],
                             start=True, stop=True)

        i0 = 0
        for c, G in enumerate(CHUNKS):
            last = c == len(CHUNKS) - 1
            N = G * WP
            Bufs = [pool.tile([P, NMAX], f16, tag=f"B{k}", name=f"B{k}")
                    for k in range(8)]
            T0, T1, T2 = Bufs[0], Bufs[1], Bufs[2]
            for g in range(G):
                s = slice(g * WP, (g + 1) * WP)
                nc.gpsimd.dma_start(T0[:, s], x[i0 + g, 0:P, :])
                nc.scalar.dma_start(T1[:, s], x[i0 + g, 1:P + 1, :])
                nc.sync.dma_start(T2[:, s], x[i0 + g, 2:P + 2, :])

            if False:
                # head chunk: pure-DVE 18-op network -- no cross-engine
                # (PE/ACT) dependencies while the input DMA is still
                # streaming in, so the DVE pipeline never stalls on the
                # M_/mid zigzag before other chunks exist to fill it.
                m_, M_ = Bufs[3], Bufs[4]
                tt(m_[:, 0:N], T0[:, 0:N], T1[:, 0:N], mn)
                tt(M_[:, 0:N], T0[:, 0:N], T1[:, 0:N], mx)
                lo, mm, hi = Bufs[0], Bufs[1], Bufs[5]
                tt(lo[:, 0:N], m_[:, 0:N], T2[:, 0:N], mn)
                tt(mm[:, 0:N], M_[:, 0:N], T2[:, 0:N], mn)
                tt(hi[:, 0:N], M_[:, 0:N], T2[:, 0:N], mx)
                mid = Bufs[2]
                tt(mid[:, 0:N], m_[:, 0:N], mm[:, 0:N], mx)
                pa, A = Bufs[3], Bufs[7]
                tt(pa[:, 0:N - 1], lo[:, 0:N - 1], lo[:, 1:N], mx)
                tt(A[:, 0:N - 2], pa[:, 0:N - 2], lo[:, 2:N], mx)
                pc, Cm = Bufs[4], Bufs[1]
                tt(pc[:, 0:N - 1], hi[:, 0:N - 1], hi[:, 1:N], mn)
                tt(Cm[:, 0:N - 2], pc[:, 0:N - 2], hi[:, 2:N], mn)
                pm, pM = Bufs[0], Bufs[4]
                tt(pm[:, 0:N - 1], mid[:, 0:N - 1], mid[:, 1:N], mn)
                tt(pM[:, 0:N - 1], mid[:, 0:N - 1], mid[:, 1:N], mx)
                t2, Bm = Bufs[5], Bufs[3]
                tt(t2[:, 0:N - 2], pM[:, 0:N - 2], mid[:, 2:N], mn)
                tt(Bm[:, 0:N - 2], pm[:, 0:N - 2], t2[:, 0:N - 2], mx)
                m1, M1 = Bufs[2], Bufs[4]
                t3, out = Bufs[0], Bufs[5]
            else:
                # T0/T1/T2 stay live until mid's PE terms read them.
                m_, M_, lo = Bufs[3], Bufs[4], Bufs[5]
                tt(m_[:, 0:N], T0[:, 0:N], T1[:, 0:N], mn)
                pe_sum(M_, N, [(1, T0), (1, T1), (-1, m_)], bank_base=0)
                tt(lo[:, 0:N], m_[:, 0:N], T2[:, 0:N], mn)
                # A-branch needs only lo: keeps DVE busy while PE/ACT make M_.
                pa, A = Bufs[3], Bufs[7]      # m_ dead after lo + M_ PE term
                tt(pa[:, 0:N - 1], lo[:, 0:N - 1], lo[:, 1:N], mx)
                tt(A[:, 0:N - 2], pa[:, 0:N - 2], lo[:, 2:N], mx)
                hi = Bufs[3]                  # pa dead after A
                tt(hi[:, 0:N], M_[:, 0:N], T2[:, 0:N], mx)
                mid = Bufs[6]
                pe_sum(mid, N, [(1, T0), (1, T1), (1, T2), (-1, lo), (-1, hi)],
                       bank_base=4)

                pc, Cm = Bufs[0], Bufs[1]     # T0, T1 dead after mid PE terms
                tt(pc[:, 0:N - 1], hi[:, 0:N - 1], hi[:, 1:N], mn)
                pe_keepalive(pc)
                tt(Cm[:, 0:N - 2], pc[:, 0:N - 2], hi[:, 2:N], mn)
                pm, pM = Bufs[2], Bufs[0]     # T2 dead; pc dead after Cm
                tt(pm[:, 0:N - 1], mid[:, 0:N - 1], mid[:, 1:N], mn)
                pe_keepalive(pm)
                tt(pM[:, 0:N - 1], mid[:, 0:N - 1], mid[:, 1:N], mx)
                t2, Bm = Bufs[5], Bufs[4]     # lo dead after pa/A + mid PE; M_ dead after hi
                tt(t2[:, 0:N - 2], pM[:, 0:N - 2], mid[:, 2:N], mn)
                pe_keepalive(t2)
                tt(Bm[:, 0:N - 2], pm[:, 0:N - 2], t2[:, 0:N - 2], mx)
                pe_keepalive(Bm)

                m1, M1 = Bufs[6], Bufs[2]     # mid dead after t2; pm dead
                t3, out = Bufs[3], Bufs[5]    # hi dead after Cm; t2 dead
            if last:
                for g in range(G):
                    s = slice(g * WP, g * WP + W)
                    tt(m1[:, s], A[:, s], Bm[:, s], mn)
                    tt(M1[:, s], A[:, s], Bm[:, s], mx)
                    tt(t3[:, s], M1[:, s], Cm[:, s], mn)
                    tt(out[:, s], m1[:, s], t3[:, s], mx)
                    eng = (nc.gpsimd, nc.scalar, nc.sync)[g % 3]
                    eng.dma_start(y[i0 + g], out[:, s])
            else:
                tt(m1[:, 0:N - 2], A[:, 0:N - 2], Bm[:, 0:N - 2], mn)
                tt(M1[:, 0:N - 2], A[:, 0:N - 2], Bm[:, 0:N - 2], mx)
                pe_keepalive(M1)
                tt(t3[:, 0:N - 2], M1[:, 0:N - 2], Cm[:, 0:N - 2], mn)
                tt(out[:, 0:N - 2], m1[:, 0:N - 2], t3[:, 0:N - 2], mx)
                pe_keepalive(out)
                for g in range(G):
                    eng = (nc.gpsimd, nc.scalar, nc.sync)[g % 3]
                    eng.dma_start(y[i0 + g], out[:, g * WP: g * WP + W])
            i0 += G

    nc.compile()
    return nc


def _get_program():
    global _PROGRAM
    if _PROGRAM is None:
        _PROGRAM = _build_program()
    return _PROGRAM


def kernel(x):
    global LAST_RESULT
    from concourse.bass_utils import run_bass_kernel_spmd
    import os

    x16 = np.asarray(x).astype(np.float16).reshape(NIMG, H, W)
    xp = np.pad(x16, ((0, 0), (1, 1), (1, 1)), mode="reflect")
    wid = np.stack([np.eye(128, dtype=np.float16),
                    -np.eye(128, dtype=np.float16)])
    in_maps = [
        {"x": np.ascontiguousarray(
            xp[:, ROWS_PER_CORE * k: ROWS_PER_CORE * k + HP_CORE, :]),
         "wid": wid}
        for k in range(NCORES)
    ]
    nc = _get_program()
    trace = bool(int(os.environ.get("MEDIAN_TRACE", "0")))
    res = run_bass_kernel_spmd(nc, in_maps, list(range(NCORES)), trace=trace)
    LAST_RESULT = res
    out = np.concatenate([res.results[k]["y"] for k in range(NCORES)], axis=1)
    return out.reshape(B, C, H, W).astype(np.float32)


# revision 12
# speedup vs baseline: 1.0226x; 1.0149x over previous
"""3x3 median filter (reflect padding) on Trainium2, 8-core data parallel.

Input  x: (4, 3, 1024, 1024) float32
Output  : (4, 3, 1024, 1024) float32  (Kornia MedianBlur semantics)

Strategy:
  - Host: convert to fp16 (DVE tensor_tensor runs 2 elem/cycle on 16-bit
    data; max err ~2^-11 << 2e-2 tolerance), reflect-pad H and W by 1,
    shard H across 8 cores with 1-row halo.
  - Device (per core): separable median network.  The vector engine is
    the only engine with tensor-tensor min/max, so it is the bottleneck;
    two of the 18 ops/pixel are offloaded using sum identities computed
    on the idle tensor + scalar engines:
        M   = max(T0,T1) = T0 + T1 - min(T0,T1)
        mid = med3(col)  = T0 + T1 + T2 - lo - hi
    PE accumulates the sums via +/-identity matmuls into PSUM (512-wide
    strips), ACT copies PSUM -> SBUF fp16.  DVE does the remaining 15
    min/max ops per pixel.
  - Images processed in chunks [2,4,4,2] concatenated along the free dim
    (amortizes the ~150-cycle/op overhead; junk junction columns never
    read).  DVE ops emitted in a stall-minimizing order (lo/pa/A before
    hi, which waits on the ACT-produced M).
"""

import sys

sys.path.insert(0, "/opt/trn_rl_repo")

import numpy as np

B, C, H, W = 4, 3, 1024, 1024
NIMG = B * C            # 12
NCORES = 8
ROWS_PER_CORE = H // NCORES   # 128
WP = W + 2              # 1026 padded width
HP_CORE = ROWS_PER_CORE + 2   # 130 padded rows per core
CHUNKS = (2, 4, 4, 2)   # images per chunk, sum = NIMG
GMAX = max(CHUNKS)
NMAX = GMAX * WP        # 4104

_PROGRAM = None
LAST_RESULT = None


def _build_program():
    import concourse.bacc as bacc
    import concourse.tile as tile
    import concourse.mybir as mybir
    from concourse.bass import MemorySpace
    from contextlib import ExitStack

    f16 = mybir.dt.float16
    f32 = mybir.dt.float32
    mn = mybir.AluOpType.min
    mx = mybir.AluOpType.max
    COPYF = mybir.ActivationFunctionType.Copy

    nc = bacc.Bacc("TRN2", target_bir_lowering=False, debug=False,
                   num_devices=NCORES)
    x = nc.dram_tensor("x", [NIMG, HP_CORE, WP], f16, kind="ExternalInput").ap()
    wid = nc.dram_tensor("wid", [2, 128, 128], f16, kind="ExternalInput").ap()
    y = nc.dram_tensor("y", [NIMG, ROWS_PER_CORE, W], f16,
                       kind="ExternalOutput").ap()

    P = ROWS_PER_CORE  # 128 partitions

    with tile.TileContext(nc) as tc, ExitStack() as ctx:
        pool = ctx.enter_context(tc.tile_pool(name="p", bufs=2))
        cpool = ctx.enter_context(tc.tile_pool(name="c", bufs=1))
        psum = ctx.enter_context(
            tc.tile_pool(name="ps", bufs=1, space=MemorySpace.PSUM))

        Iw = cpool.tile([P, P], f16, tag="Iw")
        NIw = cpool.tile([P, P], f16, tag="NIw")
        nc.sync.dma_start(Iw[:], wid[0])
        nc.sync.dma_start(NIw[:], wid[1])

        def tt(dst, a, b, op):
            nc.vector.tensor_tensor(dst, a, b, op=op)

        def pe_sum(dst_sbuf, n, terms, bank_base):
            """dst_sbuf[:, 0:n] = sum(sign*t for sign, t in terms), via PE
            accumulation in 512-wide PSUM strips + ACT copy to SBUF fp16."""
            strips = [(s, min(512, n - s)) for s in range(0, n, 512)]
            for si, (s, w) in enumerate(strips):
                bank = (bank_base + si) % 8
                ps = psum.tile([P, 512], f32, tag=f"ps{bank}", name=f"ps{bank}")
                for ti, (sign, t) in enumerate(terms):
                    nc.tensor.matmul(ps[:, 0:w], (Iw if sign > 0 else NIw)[:],
                                     t[:, s:s + w],
                                     start=(ti == 0), stop=(ti == len(terms) - 1))
                nc.scalar.activation(dst_sbuf[:, s:s + w], ps[:, 0:w], COPYF)

        i0 = 0
        for c, G in enumerate(CHUNKS):
            last = c == len(CHUNKS) - 1
            N = G * WP
            Bufs = [pool.tile([P, NMAX], f16, tag=f"B{k}", name=f"B{k}")
                    for k in range(8)]
            T0, T1, T2 = Bufs[0], Bufs[1], Bufs[2]
            for g in range(G):
                s = slice(g * WP, (g + 1) * WP)
                nc.gpsimd.dma_start(T0[:, s], x[i0 + g, 0:P, :])
                nc.scalar.dma_start(T1[:, s], x[i0 + g, 1:P + 1, :])
                nc.sync.dma_start(T2[:, s], x[i0 + g, 2:P + 2, :])

            # T0/T1/T2 stay live until mid's PE terms read them.
            m_, M_, lo = Bufs[3], Bufs[4], Bufs[5]
            tt(m_[:, 0:N], T0[:, 0:N], T1[:, 0:N], mn)
            pe_sum(M_, N, [(1, T0), (1, T1), (-1, m_)], bank_base=0)
            tt(lo[:, 0:N], m_[:, 0:N], T2[:, 0:N], mn)
            # A-branch needs only lo: keeps DVE busy while PE/ACT make M_.
            pa, A = Bufs[3], Bufs[7]      # m_ dead after lo + M_ PE term
            tt(pa[:, 0:N - 1], lo[:, 0:N - 1], lo[:, 1:N], mx)
            tt(A[:, 0:N - 2], pa[:, 0:N - 2], lo[:, 2:N], mx)
            hi = Bufs[3]                  # pa dead after A
            tt(hi[:, 0:N], M_[:, 0:N], T2[:, 0:N], mx)
            mid = Bufs[6]
            pe_sum(mid, N, [(1, T0), (1, T1), (1, T2), (-1, lo), (-1, hi)],
                   bank_base=4)

            pc, Cm = Bufs[0], Bufs[1]     # T0, T1 dead after mid PE terms
            tt(pc[:, 0:N - 1], hi[:, 0:N - 1], hi[:, 1:N], mn)
            tt(Cm[:, 0:N - 2], pc[:, 0:N - 2], hi[:, 2:N], mn)
            pm, pM = Bufs[2], Bufs[0]     # T2 dead; pc dead after Cm
            tt(pm[:, 0:N - 1], mid[:, 0:N - 1], mid[:, 1:N], mn)
            tt(pM[:, 0:N - 1], mid[:, 0:N - 1], mid[:, 1:N], mx)
            t2, Bm = Bufs[5], Bufs[4]     # lo dead after pa/A + mid PE; M_ dead after hi
            tt(t2[:, 0:N - 2], pM[:, 0:N - 2], mid[:, 2:N], mn)
            tt(Bm[:, 0:N - 2], pm[:, 0:N - 2], t2[:, 0:N - 2], mx)

            m1, M1 = Bufs[6], Bufs[2]     # mid dead after t2; pm dead
            t3, out = Bufs[3], Bufs[5]    # hi dead after Cm; t2 dead
            if last:
                for g in range(G):
                    s = slice(g * WP, g * WP + W)
                    tt(m1[:, s], A[:, s], Bm[:, s], mn)
                    tt(M1[:, s], A[:, s], Bm[:, s], mx)
                    tt(t3[:, s], M1[:, s], Cm[:, s], mn)
                    tt(out[:, s], m1[:, s], t3[:, s], mx)
                    eng = (nc.gpsimd, nc.scalar, nc.sync)[g % 3]
                    eng.dma_start(y[i0 + g], out[:, s])
            else:
                tt(m1[:, 0:N - 2], A[:, 0:N - 2], Bm[:, 0:N - 2], mn)
                tt(M1[:, 0:N - 2], A[:, 0:N - 2], Bm[:, 0:N - 2], mx)
                tt(t3[:, 0:N - 2], M1[:, 0:N - 2], Cm[:, 0:N - 2], mn)
                tt(out[:, 0:N - 2], m1[:, 0:N - 2], t3[:, 0:N - 2], mx)
                for g in range(G):
                    eng = (nc.gpsimd, nc.scalar, nc.sync)[g % 3]
                    eng.dma_start(y[i0 + g], out[:, g * WP: g * WP + W])
            i0 += G

    nc.compile()
    return nc


def _get_program():
    global _PROGRAM
    if _PROGRAM is None:
        _PROGRAM = _build_program()
    return _PROGRAM


def kernel(x):
    global LAST_RESULT
    from concourse.bass_utils import run_bass_kernel_spmd
    import os

    x16 = np.asarray(x).astype(np.float16).reshape(NIMG, H, W)
    xp = np.pad(x16, ((0, 0), (1, 1), (1, 1)), mode="reflect")
    wid = np.stack([np.eye(128, dtype=np.float16),
                    -np.eye(128, dtype=np.float16)])
    in_maps = [
        {"x": np.ascontiguousarray(
            xp[:, ROWS_PER_CORE * k: ROWS_PER_CORE * k + HP_CORE, :]),
         "wid": wid}
        for k in range(NCORES)
    ]
    nc = _get_program()
    trace = bool(int(os.environ.get("MEDIAN_TRACE", "0")))
    res = run_bass_kernel_spmd(nc, in_maps, list(range(NCORES)), trace=trace)
    LAST_RESULT = res
    out = np.concatenate([res.results[k]["y"] for k in range(NCORES)], axis=1)
    return out.reshape(B, C, H, W).astype(np.float32)

